# revision 7
# baseline (speedup 1.0000x reference)
"""Trainium2 Bass kernel for nn_CartesianProductClassifier.

out[b,i,j] = sigmoid(MLP(concat(x[b,j], x[b,i])))  for x [8, 512, 32].

Math restructuring:
  layer1: h1[b,i,j] = relu(A[b,j] + C[b,i])   with A = x@W1_top (N-sized),
          C = x@W1_bot + b1 (N-sized)  -> first layer is O(N), not O(N^2).
  layer2 runs as FOUR concurrent 64x64 quadrant matmuls (tile_position) so
  one 512-cycle pass covers both i-rows and both packed batches; its output
  lands batch-packed in one 2-bank PSUM tile so a single FD=1024 scalar
  activation does the relu.  layer3 is two col-grouped matmuls; layer4 uses
  column-shifted W4 accumulating all 32 twins of a batch-pair into one PSUM
  bank.
Engine balance per twin: DVE generates h1a/h1b + evicts h3, Scalar evicts
h2 (~1.1us/twin each).

Sharding: core c handles rows i in [64c, 64c+64) of all 8 batches.
"""

import numpy as np

B, N, D = 8, 512, 32
NCORES = 8
RPC = N // NCORES  # rows per core = 64

_PROG = None


def _build_program():
    import concourse.mybir as mybir
    import concourse.tile as tile
    from concourse import bacc

    dt = mybir.dt
    F32 = dt.float32
    BF16 = dt.bfloat16
    AF = mybir.ActivationFunctionType
    OP = mybir.AluOpType

    nc = bacc.Bacc(
        "TRN2", target_bir_lowering=False, debug=False, num_devices=NCORES
    )

    # xT2: [64, 4*512] col = bp*512 + j; rows 0:32 features of batch 2bp,
    #      rows 32:64 of batch 2bp+1.   xcT2: same layout, col = bp*64 + i.
    xT2 = nc.dram_tensor("xT2", [2 * D, 4 * N], BF16, kind="ExternalInput")
    # wcomb packs [xcT2 (256) | w1tbd (128) | w1bbd (128)] on 64 partitions
    wcomb = nc.dram_tensor("wcomb", [2 * D, 512], BF16, kind="ExternalInput")
    # w23 packs [w2st (64) | w3bd (64)] on 128 partitions
    w23 = nc.dram_tensor("w23", [128, 128], BF16, kind="ExternalInput")
    w4sh = nc.dram_tensor("w4sh", [128, 32 * 128], BF16, kind="ExternalInput")
    # bcomb packs [b1s | b2s | b3s | b4s] columns
    bcomb = nc.dram_tensor("bcomb", [128, 4], F32, kind="ExternalInput")
    out = nc.dram_tensor("out", [B, RPC, N], F32, kind="ExternalOutput")

    with tile.TileContext(nc) as tc:
        with (
            tc.tile_pool(name="const", bufs=1) as const,
            tc.tile_pool(name="h1p", bufs=8) as h1p,
            tc.tile_pool(name="h2p", bufs=6) as h2p,
            tc.tile_pool(name="h3p", bufs=6) as h3p,
            tc.tile_pool(name="sigp", bufs=2) as sigp,
            tc.tile_pool(name="psA", bufs=3, space="PSUM") as psA,
            tc.tile_pool(name="psC", bufs=2, space="PSUM") as psC,
        ):
            # ---------- constant loads ----------
            # xT2 first on sync (critical path for stage A)
            xT2_sb = const.tile([2 * D, 4 * N], BF16, tag="xT2")
            nc.sync.dma_start(xT2_sb[:], xT2[:])
            wcomb_sb = const.tile([2 * D, 512], BF16, tag="wcomb")
            nc.sync.dma_start(wcomb_sb[:], wcomb[:])
            w23_sb = const.tile([128, 128], BF16, tag="w23")
            nc.sync.dma_start(w23_sb[:], w23[:])
            bcomb_sb = const.tile([128, 4], F32, tag="bcomb")
            nc.sync.dma_start(bcomb_sb[:], bcomb[:])
            xcT2_sb = wcomb_sb[:, 0:256]
            w1tbd_sb = wcomb_sb[:, 256:384]
            w1bbd_sb = wcomb_sb[:, 384:512]
            w2st_sb = w23_sb[:, 0:64]
            w3bd_sb = w23_sb[:, 64:128]
            b1s_sb = bcomb_sb[:, 0:1]
            b2s_sb = bcomb_sb[:, 1:2]
            b3s_sb = bcomb_sb[:, 2:3]
            b4s_sb = bcomb_sb[:, 3:4]
            w4sh_sb = const.tile([128, 32 * 128], BF16, tag="w4sh")
            for q in range(4):
                # chunk q only gates mm4 twins 8q..8q+7
                nc.sync.dma_start(
                    w4sh_sb[:, q * 1024 : (q + 1) * 1024],
                    w4sh[:, q * 1024 : (q + 1) * 1024],
                )

            # ---------- stage A: AT2 [128, 2048], Cpp [128, 256] ----------
            # AT2[:, bp*512 + j]: p<64 -> A-hidden of batch 2bp at col j,
            #                     p>=64 -> batch 2bp+1.
            AT2 = const.tile([128, 4 * N], BF16, tag="AT2")
            Cpp = const.tile([128, 4 * RPC], F32, tag="Cpp")
            for bp in range(4):
                psa = psA.tile([128, N], F32, tag="ps2", name=f"psa{bp}")
                nc.tensor.matmul(
                    psa[:],
                    w1tbd_sb,
                    xT2_sb[:, bp * N : (bp + 1) * N],
                    start=True,
                    stop=True,
                )
                nc.vector.tensor_copy(AT2[:, bp * N : (bp + 1) * N], psa[:])
            psc = psA.tile([128, N], F32, tag="ps2", name="psc")
            nc.tensor.matmul(
                psc[:, 0 : 4 * RPC],
                w1bbd_sb,
                xcT2_sb,
                start=True,
                stop=True,
            )
            # Cpp = psc + b1 (no relu here; relu happens after adding A)
            nc.vector.tensor_scalar(
                Cpp[:], psc[:, 0 : 4 * RPC], b1s_sb, None, OP.add
            )

            # ---------- main loop: 128 twins, software-pipelined ----------
            # Twin g covers i-rows {2t, 2t+1} of batch-pair bp (g = 32*bp+t).
            # Emission skew per g: h1(g+2) | mm2+h2(g+1) | mm3+h3+mm4(g)
            # so DVE's strict FIFO never queues h3(g) ahead of h1 work that
            # is ready to run (v2 stalled ~2.7us every 3rd twin on that).
            NT = 128
            h1_tiles = {}
            pq_tiles = {}
            h2_tiles = {}
            sig_tiles = {}

            def emit_h1(g):
                bp, t = g >> 5, g & 31
                at = AT2[:, bp * N : (bp + 1) * N]
                pair = []
                for half in range(2):
                    i = 2 * t + half
                    h1 = h1p.tile(
                        [128, N], BF16, tag="h1", name=f"h1_{g}_{half}"
                    )
                    nc.vector.tensor_scalar(
                        h1[:],
                        at,
                        Cpp[:, bp * RPC + i : bp * RPC + i + 1],
                        0.0,
                        OP.add,
                        OP.max,
                    )
                    pair.append(h1)
                h1_tiles[g] = pair

            def emit_l2(g):
                h1a, h1b = h1_tiles.pop(g)
                # four concurrent 64x64 quadrant matmuls:
                # bank0 (cols 0:N)  = batch-even [z2(ia); z2(ib)]
                # bank1 (cols N:2N) = batch-odd  [z2(ia); z2(ib)]
                pq = psA.tile([128, 2 * N], F32, tag="ps2", name=f"pq_{g}")
                nc.tensor.matmul(
                    pq[0:64, 0:N],
                    w2st_sb[0:64, :],
                    h1a[0:64, :],
                    start=True,
                    stop=True,
                    tile_position=(0, 0),
                )
                nc.tensor.matmul(
                    pq[0:64, N : 2 * N],
                    w2st_sb[64:128, :],
                    h1a[64:128, :],
                    start=True,
                    stop=True,
                    tile_position=(64, 0),
                    skip_group_check=True,
                )
                nc.tensor.matmul(
                    pq[64:128, 0:N],
                    w2st_sb[0:64, :],
                    h1b[0:64, :],
                    start=True,
                    stop=True,
                    tile_position=(0, 64),
                    skip_group_check=True,
                )
                nc.tensor.matmul(
                    pq[64:128, N : 2 * N],
                    w2st_sb[64:128, :],
                    h1b[64:128, :],
                    start=True,
                    stop=True,
                    tile_position=(64, 64),
                    skip_group_check=True,
                )
                # h2 relu: one scalar activation covers both banks
                h2pq = h2p.tile([128, 2 * N], BF16, tag="h2", name=f"h2_{g}")
                nc.scalar.activation(
                    h2pq[:], pq[:], AF.Relu, bias=b2s_sb, scale=1.0
                )
                pq_tiles[g] = pq
                h2_tiles[g] = h2pq

            def emit_l34(g):
                bp, t = g >> 5, g & 31
                pq = pq_tiles.pop(g)
                h2pq = h2_tiles.pop(g)
                # layer 3: two col-grouped matmuls (concurrent); output
                # reuses bank0 of this twin's own pq slot (WAR on the h2
                # activation, which mm3 depends on anyway).
                # ps3[0:64]  = batch-even [z3(ia) 0:32; z3(ib) 32:64]
                # ps3[64:128]= batch-odd
                ps3 = pq[:, 0:N]
                nc.tensor.matmul(
                    ps3[0:64, :],
                    w3bd_sb,
                    h2pq[:, 0:N],
                    start=True,
                    stop=True,
                    skip_group_check=True,
                )
                nc.tensor.matmul(
                    ps3[64:128, :],
                    w3bd_sb,
                    h2pq[:, N : 2 * N],
                    start=True,
                    stop=True,
                    skip_group_check=True,
                )
                h3 = h3p.tile([128, N], BF16, tag="h3", name=f"h3_{g}")
                nc.vector.tensor_scalar(
                    h3[:], ps3, b3s_sb, 0.0, OP.add, OP.max
                )
                # layer 4: W4 shifted to columns -> rows of sig_ps;
                # accumulate all 32 twins of this bp into one PSUM bank
                if t == 0:
                    sig_tiles[bp] = psC.tile(
                        [128, N], F32, tag="sig", name=f"sig{bp}"
                    )
                nc.tensor.matmul(
                    sig_tiles[bp][:],
                    w4sh_sb[:, t * 128 : (t + 1) * 128],
                    h3[:],
                    start=(t == 0),
                    stop=(t == 31),
                    skip_group_check=True,
                )
                if t == 31:
                    sig_sb = sigp.tile(
                        [128, N], F32, tag="sig_sb", name=f"sigsb{bp}"
                    )
                    nc.scalar.activation(
                        sig_sb[:],
                        sig_tiles.pop(bp)[:],
                        AF.Sigmoid,
                        bias=b4s_sb,
                        scale=1.0,
                    )
                    nc.sync.dma_start(out[2 * bp, :, :], sig_sb[0:64, :])
                    nc.sync.dma_start(out[2 * bp + 1, :, :], sig_sb[64:128, :])

            emit_h1(0)
            emit_h1(1)
            emit_l2(0)
            for g in range(NT):
                if g + 2 < NT:
                    emit_h1(g + 2)
                if g + 1 < NT:
                    emit_l2(g + 1)
                emit_l34(g)

    nc.compile()
    return nc


def _get_program():
    global _PROG
    if _PROG is None:
        _PROG = _build_program()
    return _PROG


def prep_in_maps(inputs):
    import ml_dtypes

    x = np.ascontiguousarray(np.asarray(inputs["x"], dtype=np.float32))
    W1 = np.asarray(inputs["W1"], dtype=np.float32)
    b1 = np.asarray(inputs["b1"], dtype=np.float32)
    W2 = np.asarray(inputs["W2"], dtype=np.float32)
    b2 = np.asarray(inputs["b2"], dtype=np.float32)
    W3 = np.asarray(inputs["W3"], dtype=np.float32)
    b3 = np.asarray(inputs["b3"], dtype=np.float32)
    W4 = np.asarray(inputs["W4"], dtype=np.float32)
    b4 = np.asarray(inputs["b4"], dtype=np.float32)

    bf16 = ml_dtypes.bfloat16
    w1tbd = np.zeros((2 * D, 128), bf16)
    w1tbd[:D, :64] = W1[:D].astype(bf16)
    w1tbd[D:, 64:] = W1[:D].astype(bf16)
    w1bbd = np.zeros((2 * D, 128), bf16)
    w1bbd[:D, :64] = W1[D:].astype(bf16)
    w1bbd[D:, 64:] = W1[D:].astype(bf16)
    # w23: [w2st | w3bd].  w2st = W2 stacked on both partition halves
    # (each 64x64 quadrant matmul reads its own half).  w3bd block-diag
    # maps [h2(i-even) feats; h2(i-odd) feats] -> [z3(i-even); z3(i-odd)].
    w23 = np.zeros((128, 128), bf16)
    w23[:64, 0:64] = W2.astype(bf16)
    w23[64:, 0:64] = W2.astype(bf16)
    w23[:64, 64:96] = W3.astype(bf16)
    w23[64:, 96:128] = W3.astype(bf16)
    # w4sh[t]: h3 row-block r -> output column
    #   r=0 (b-even, ia) -> 2t      r=1 (b-even, ib) -> 2t+1
    #   r=2 (b-odd,  ia) -> 64+2t   r=3 (b-odd,  ib) -> 64+2t+1
    w4sh = np.zeros((128, 32 * 128), bf16)
    w4c = W4[:, 0].astype(bf16)
    for t in range(32):
        cols = (2 * t, 2 * t + 1, 64 + 2 * t, 64 + 2 * t + 1)
        for rr in range(4):
            w4sh[32 * rr : 32 * rr + 32, t * 128 + cols[rr]] = w4c

    # xT2 [64, 4*512]: col = bp*512 + j, rows 0:32 batch 2bp, 32:64 batch 2bp+1
    xT2 = np.zeros((2 * D, 4 * N), bf16)
    for bp in range(4):
        xT2[:D, bp * N : (bp + 1) * N] = x[2 * bp].T.astype(bf16)
        xT2[D:, bp * N : (bp + 1) * N] = x[2 * bp + 1].T.astype(bf16)

    bcomb = np.zeros((128, 4), np.float32)
    bcomb[:, 0] = np.concatenate([b1, b1])
    bcomb[:, 1] = np.concatenate([b2, b2])
    bcomb[:, 2] = np.tile(b3, 4)
    bcomb[:, 3] = b4[0]

    common = {
        "xT2": xT2,
        "w23": w23,
        "w4sh": w4sh,
        "bcomb": bcomb,
    }
    in_maps = []
    for c in range(NCORES):
        xc = x[:, c * RPC : (c + 1) * RPC, :]  # [8, 64, 32]
        wcomb = np.zeros((2 * D, 512), bf16)
        for bp in range(4):
            wcomb[:D, bp * RPC : (bp + 1) * RPC] = xc[2 * bp].T.astype(bf16)
            wcomb[D:, bp * RPC : (bp + 1) * RPC] = xc[2 * bp + 1].T.astype(bf16)
        wcomb[:, 256:384] = w1tbd
        wcomb[:, 384:512] = w1bbd
        in_maps.append({**common, "wcomb": wcomb})
    return in_maps


def assemble(per_core_outs):
    full = np.empty((B, N, N), np.float32)
    for c in range(NCORES):
        full[:, c * RPC : (c + 1) * RPC, :] = per_core_outs[c]
    return full


def run(inputs, trace=False):
    """Returns (full_output, BassKernelResults)."""
    from concourse.bass_utils import run_bass_kernel_spmd

    nc = _get_program()
    in_maps = prep_in_maps(inputs)
    res = run_bass_kernel_spmd(nc, in_maps, list(range(NCORES)), trace=trace)
    full = assemble([res.results[c]["out"] for c in range(NCORES)])
    return full, res


def kernel(**inputs):
    full, _ = run(inputs, trace=False)
    return full
